# revision 1
# baseline (speedup 1.0000x reference)
"""CharAttention TRN2 kernel: 8-core data-parallel over words.

Only the LAST valid character's attention output is consumed by the
reference, so per word we compute
  q      = x[last] @ Wq                 (one query per word)
  K, V   = x @ Wk, x @ Wv               (all 32 positions)
  scores = q . K / sqrt(hd), masked to j <= last
  o      = softmax(scores) @ V
  out    = (o + pos) @ Wp
which drops 1/3 of the projection FLOPs and the full c x c attention.

Distribution: 1024 words split 128/core across 8 cores (pure data
parallel, no collectives). All matmuls run as float32r (TF32-like,
1 cycle/row at N=512). x is uploaded pre-transposed [C, T]; K^T/V^T
tiles [(2 heads x 64 d), tokens] are consumed directly out of PSUM;
the d-contraction of q.K and the head-expansion of softmax(p) are done
on the PE with small constant matrices so the DVE only does
elementwise work.
"""
import os
import numpy as np

B, W, CC, C = 4, 256, 32, 1024
H, HD = 16, 64
NCORES = 8
WPC = (B * W) // NCORES          # 128 words per core
T = WPC * CC                     # 4096 tokens per core
NE = 8                           # e-tiles (contraction over C)
NF = 8                           # f-tiles per 1024-wide projection
RND = 4                          # token rounds per f-tile
TR = T // RND                    # 1024 tokens per round
WR = TR // CC                    # 32 words per round

_cache = {}
LAST_EXEC_NS = None


def _build_nc():
    import concourse.mybir as mybir
    import concourse.tile as tile
    from concourse import bacc

    f32 = mybir.dt.float32
    f32r = mybir.dt.float32r
    bf16 = mybir.dt.bfloat16
    Exp = mybir.ActivationFunctionType.Exp
    AX = mybir.AxisListType.X

    nc = bacc.Bacc("TRN2", target_bir_lowering=False, num_devices=NCORES,
                   debug=False)

    xT = nc.declare_dram_parameter("xT", [NE, 128, T], f32r, isOutput=False)
    xlT = nc.declare_dram_parameter("xlT", [128, C], f32r, isOutput=False)
    wq_d = nc.declare_dram_parameter("wq_d", [NF, 128, C], f32r, isOutput=False)
    wk_d = nc.declare_dram_parameter("wk_d", [NF, 128, C], f32r, isOutput=False)
    wv_d = nc.declare_dram_parameter("wv_d", [NF, 128, C], f32r, isOutput=False)
    wp_d = nc.declare_dram_parameter("wp_d", [NF, 128, C], f32r, isOutput=False)
    posT = nc.declare_dram_parameter("posT", [128, C], f32, isOutput=False)
    mask_in = nc.declare_dram_parameter("mask_in", [16, T], bf16, isOutput=False)
    e16_in = nc.declare_dram_parameter("e16_in", [16, C], f32r, isOutput=False)
    hw2_in = nc.declare_dram_parameter("hw2_in", [128, 32], f32r, isOutput=False)
    out = nc.declare_dram_parameter("out", [128, C], f32, isOutput=True)

    with tile.TileContext(nc) as tc:
        with tc.tile_pool(name="big", bufs=1) as big, \
             tc.tile_pool(name="wpool", bufs=2) as wpool, \
             tc.tile_pool(name="work", bufs=2) as work, \
             tc.tile_pool(name="small", bufs=1) as small, \
             tc.tile_pool(name="psA", bufs=3, space="PSUM") as psA, \
             tc.tile_pool(name="psS", bufs=2, space="PSUM") as psS:

            # ---- resident loads (small inputs first so Q starts early) ----
            xlT_sb = small.tile([128, C], f32r, tag="scrA")
            nc.sync.dma_start(xlT_sb[:], xlT[:])
            hw2_sb = small.tile([128, 32], f32r)
            nc.sync.dma_start(hw2_sb[:], hw2_in[:])
            e16_sb = small.tile([16, C], f32r)
            nc.sync.dma_start(e16_sb[:], e16_in[:])
            posT_sb = small.tile([128, C], f32)
            nc.sync.dma_start(posT_sb[:], posT[:])
            mask_sb = small.tile([16, T], bf16)
            nc.sync.dma_start(mask_sb[:], mask_in[:])

            qT_sb = small.tile([128, C], f32, tag="scrB")
            scores = small.tile([16, T], f32r)
            s_sb = small.tile([16, WPC], f32)

            # ---- Q projection: qT[f, w] accumulated over e-tiles ----
            for i in range(NF):
                wq_t = wpool.tile([128, C], f32r, tag="w")
                nc.sync.dma_start(wq_t[:], wq_d[i])
                psq = psA.tile([128, 128], f32, tag="psA")
                for t in range(NE):
                    nc.tensor.matmul(
                        psq[:], wq_t[:, t * 128:(t + 1) * 128],
                        xlT_sb[:, t * 128:(t + 1) * 128],
                        start=(t == 0), stop=(t == NE - 1))
                nc.any.tensor_copy(qT_sb[:, i * 128:(i + 1) * 128], psq[:])

            # x^T: one tile per e-tile, split across both HWDGE engines,
            # issued after the q-phase weight loads so Q starts immediately
            xts = []
            for t in range(NE):
                xt = big.tile([128, T], f32r, tag=f"xt{t}")
                eng = nc.sync if t % 2 == 0 else nc.scalar
                eng.dma_start(xt[:], xT[t])
                xts.append(xt)


            # ---- K projection + scores ----
            for i in range(NF):
                wk_t = wpool.tile([128, C], f32r, tag="w")
                nc.sync.dma_start(wk_t[:], wk_d[i])
                for r in range(RND):
                    psk = psA.tile([128, TR], f32, tag="psA")
                    for t in range(NE):
                        for ch in range(2):
                            nc.tensor.matmul(
                                psk[:, ch * 512:(ch + 1) * 512],
                                wk_t[:, t * 128:(t + 1) * 128],
                                xts[t][:, r * TR + ch * 512:
                                       r * TR + (ch + 1) * 512],
                                start=(t == 0), stop=(t == NE - 1))
                    # prod[(2h,64d), (w,j)] = K^T * qT broadcast over j
                    prod = work.tile([128, TR], f32r, tag="prod")
                    qv = qT_sb[:, i * 128 + r * WR: i * 128 + (r + 1) * WR]
                    nc.vector.tensor_mul(
                        prod[:].rearrange("p (w j) -> p w j", j=CC),
                        psk[:].rearrange("p (w j) -> p w j", j=CC),
                        qv[:, :, None].broadcast_to([128, WR, CC]))
                    # scores[h, (w,j)] += Hsum_i.T @ prod  (sums over d)
                    for ch in range(2):
                        pss = psS.tile([16, 512], f32, tag="psS")
                        nc.tensor.matmul(
                            pss[:], hw2_sb[:, 14 - 2 * i: 30 - 2 * i],
                            prod[:, ch * 512:(ch + 1) * 512],
                            start=True, stop=True)
                        seg = slice(r * TR + ch * 512, r * TR + (ch + 1) * 512)
                        if i == 0:
                            nc.vector.tensor_copy(scores[:, seg], pss[:])
                        else:
                            nc.vector.tensor_add(scores[:, seg],
                                                 scores[:, seg], pss[:])

            # ---- masked softmax over j, segmented per round so V's
            # P_exp matmuls can start as soon as their quarter is done ----
            for r in range(RND):
                sr = scores[:, r * TR:(r + 1) * TR]
                sv = s_sb[:, r * WR:(r + 1) * WR]
                nc.scalar.activation(sr, sr, Exp,
                                     scale=1.0 / float(np.sqrt(HD)))
                nc.vector.tensor_mul(sr, sr, mask_sb[:, r * TR:(r + 1) * TR])
                nc.vector.reduce_sum(
                    sv, sr.rearrange("p (w j) -> p w j", j=CC), axis=AX)
                nc.vector.reciprocal(sv, sv)
                nc.vector.tensor_mul(
                    sr.rearrange("p (w j) -> p w j", j=CC),
                    sr.rearrange("p (w j) -> p w j", j=CC),
                    sv[:, :, None].broadcast_to([16, WR, CC]))

            oT_sb = small.tile([128, C], f32, tag="scrB")

            # ---- V projection + weighted sum over j ----
            for i in range(NF):
                wv_t = wpool.tile([128, C], f32r, tag="w")
                nc.sync.dma_start(wv_t[:], wv_d[i])
                for r in range(RND):
                    psv = psA.tile([128, TR], f32, tag="psA")
                    for t in range(NE):
                        for ch in range(2):
                            nc.tensor.matmul(
                                psv[:, ch * 512:(ch + 1) * 512],
                                wv_t[:, t * 128:(t + 1) * 128],
                                xts[t][:, r * TR + ch * 512:
                                       r * TR + (ch + 1) * 512],
                                start=(t == 0), stop=(t == NE - 1))
                    # P_exp[(2h,64d), (w,j)] = E16_i.T @ p
                    pspe = psA.tile([128, TR], f32, tag="psA")
                    for ch in range(2):
                        nc.tensor.matmul(
                            pspe[:, ch * 512:(ch + 1) * 512],
                            e16_sb[:, i * 128:(i + 1) * 128],
                            scores[:, r * TR + ch * 512:
                                   r * TR + (ch + 1) * 512],
                            start=True, stop=True)
                    pexp_sb = work.tile([128, TR], f32, tag="pexp", bufs=1)
                    nc.scalar.copy(pexp_sb[:], pspe[:])
                    prodv = work.tile([128, TR], f32, tag="prodv", bufs=1)
                    nc.vector.tensor_mul(prodv[:], psv[:], pexp_sb[:])
                    nc.vector.reduce_sum(
                        oT_sb[:, i * 128 + r * WR: i * 128 + (r + 1) * WR],
                        prodv[:].rearrange("p (w j) -> p w j", j=CC), axis=AX)

            # ---- output projection: out[w, g] = (oT + posT).T @ Wp ----
            sum_sb = small.tile([128, C], f32r, tag="scrA")
            nc.vector.tensor_add(sum_sb[:], oT_sb[:], posT_sb[:])
            pso = psA.tile([128, C], f32, tag="psA")
            for i in range(NF):
                wp_t = wpool.tile([128, C], f32r, tag="wp")
                nc.sync.dma_start(wp_t[:], wp_d[i])
                for ch in range(2):
                    nc.tensor.matmul(
                        pso[:, ch * 512:(ch + 1) * 512],
                        sum_sb[:, i * 128:(i + 1) * 128],
                        wp_t[:, ch * 512:(ch + 1) * 512],
                        start=(i == 0), stop=(i == NF - 1))
            out_sb = small.tile([128, C], f32, tag="scrC")
            nc.any.tensor_copy(out_sb[:], pso[:])
            nc.sync.dma_start(out[:], out_sb[:])

    nc.finalize()
    return nc


def _tile_lhsT(m):
    """[C, n] -> [128, (C//128)*n] device layout: row p, col t*n+j = m[t*128+p, j]."""
    n = m.shape[1]
    return np.ascontiguousarray(
        m.reshape(NE, 128, n).transpose(1, 0, 2).reshape(128, NE * n))


def _prep_inputs(x, attention_mask, pos_emb, attn_w, proj_w):
    import ml_dtypes

    x = np.asarray(x, dtype=np.float32)
    attention_mask = np.asarray(attention_mask)
    pos_emb = np.asarray(pos_emb, dtype=np.float32)
    attn_w = np.asarray(attn_w, dtype=np.float32)
    proj_w = np.asarray(proj_w, dtype=np.float32)

    x2 = x.reshape(B * W, CC, C)
    last = (attention_mask.sum(axis=2).reshape(B * W).astype(np.int64) - 1) % CC

    wq = attn_w[:, :C]
    wk = attn_w[:, C:2 * C]
    wv = attn_w[:, 2 * C:]

    def wdev(wm):  # [C, C] -> [NF, 128, C] with [i, p, t*128+f] = wm[t*128+p, i*128+f]
        return np.ascontiguousarray(
            wm.reshape(NE, 128, NF, 128).transpose(2, 1, 0, 3).reshape(NF, 128, C))

    wq_d = wdev(wq)
    wk_d = wdev(wk)
    wv_d = wdev(wv)
    wp_d = np.ascontiguousarray(proj_w.reshape(NF, 128, C))

    e16 = np.kron(np.eye(16, dtype=np.float32), np.ones((1, 64), np.float32))
    hw2 = np.zeros((128, 32), np.float32)
    hw2[np.arange(128), np.arange(128) // 64 + 14] = 1.0

    in_maps = []
    for core in range(NCORES):
        ws = slice(core * WPC, (core + 1) * WPC)
        xs = x2[ws]                                   # [128, 32, C]
        xTc = xs.reshape(T, C).T                      # [C, T]
        xT_dev = np.ascontiguousarray(xTc.reshape(NE, 128, T))
        xl = xs[np.arange(WPC), last[ws]]             # [128, C]
        xlT_dev = _tile_lhsT(np.ascontiguousarray(xl.T))
        gidx = np.arange(core * WPC, (core + 1) * WPC)
        posw = pos_emb[gidx % W]                      # [128, C]
        posT_dev = _tile_lhsT(np.ascontiguousarray(posw.T))
        maskw = (np.arange(CC)[None, :] <= last[ws][:, None])   # [128, 32]
        mask16 = np.broadcast_to(
            maskw.reshape(1, T), (16, T)).astype(ml_dtypes.bfloat16)
        in_maps.append({
            "xT": xT_dev, "xlT": xlT_dev,
            "wq_d": wq_d, "wk_d": wk_d, "wv_d": wv_d, "wp_d": wp_d,
            "posT": posT_dev, "mask_in": np.ascontiguousarray(mask16),
            "e16_in": e16, "hw2_in": hw2,
        })
    return in_maps


def kernel(x, attention_mask, pos_emb, attn_w, proj_w):
    global LAST_EXEC_NS
    from concourse.bass_utils import run_bass_kernel_spmd

    in_maps = _prep_inputs(x, attention_mask, pos_emb, attn_w, proj_w)
    if "nc" not in _cache:
        _cache["nc"] = _build_nc()
    nc = _cache["nc"]
    trace = os.environ.get("KBENCH_TRACE") == "1"
    res = run_bass_kernel_spmd(nc, in_maps, core_ids=list(range(NCORES)),
                               trace=trace)
    if trace:
        LAST_EXEC_NS = res.exec_time_ns
    _cache["last_res"] = res
    full = np.concatenate([res.results[c]["out"] for c in range(NCORES)],
                          axis=0)
    return np.ascontiguousarray(full.reshape(B, W, C).astype(np.float32))

